# revision 23
# baseline (speedup 1.0000x reference)
"""Binarized-MLP (BNN) kernel for Trainium2, data-parallel over batch on 8 NeuronCores.

Reference computation:
    h      = x @ sign(W1) + b1          x:[8192,4096] W1:[4096,512]
    logits = sign(h) @ sign(W2) + b2    W2:[512,10]
    out    = softmax(logits)            [8192,10]

Device strategy (per core, batch shard of 1024 rows):
  - Mixed-precision first matmul in ~1.5 passes instead of 2:
      hi pass: x in fp16 (11 sig bits, exact products in the PE's FP22
               pipeline), stationary sign(W1) in fp16 (+-1 exact).
      lo pass: residual r = x - fp16(x), scaled by 2^14 and cast to e4m3,
               via fp8 DoubleRow matmuls (2 contraction rows per PE cell,
               ~0.5 cycles/row). The 2^-14 descale is baked into the
               stationary operand (+-2^-14, exactly representable as an
               e5m2 NORMAL), so lo products are +-r and accumulate into
               the SAME fp32 PSUM banks as the hi pass - no combine step.
    Total x precision ~15 bits -> end-to-end rel err ~1.2e-2 (measured,
    deterministic inputs; gate is 2e-2). Sign flips of h drive the error;
    fp16-only (11 bits) measures 5e-2 and fails, hi/lo bf16 (baseline)
    measures 2.8e-4 but costs a full 2nd pass.
  - Layout: stationary = sign(W1) f-tile [128f x 128j], moving = xT f-tile
    [128f x 512b] -> PSUM [128j x 512b]; all 8 PSUM banks hold the full
    per-core h [512 x 1024] and accumulate across the 32 f-tiles.
  - Inputs host-packed so four f-tiles arrive per DMA (quads); the last
    quad runs bank-major so sign(h)/second-matmul/softmax overlap it.
  - sign(h)+b1 fused into one ScalarE Sign-activation (bias=b1) straight
    out of PSUM into bf16 SBUF tiles laid out [j, b] for the 2nd matmul.
  - Second matmul: stationary = sign(h) [128j x 128b], moving = sign(W2)
    [128j x 10] accumulated over 4 j-tiles -> PSUM [128b x 10].
  - Softmax on [128b, 10] tiles: add b2, reduce_max(negate) -> Exp with
    per-row bias and fused row-sum, Ln, subtract, Exp -> packed DMA out.
"""

import numpy as np
import ml_dtypes

import concourse.bass as bass
import concourse.bass_utils as _bass_utils
import concourse.tile as tile
from concourse import mybir
from concourse.bass_utils import run_bass_kernel_spmd
from bass_rust import ScopedClock, VectorClock

_CLEAR_SEMS = True

BF16 = mybir.dt.bfloat16
F16 = mybir.dt.float16
F32 = mybir.dt.float32
E4 = mybir.dt.float8e4
E5 = mybir.dt.float8e5
DR = mybir.MatmulPerfMode.DoubleRow

B, F, H, C = 8192, 4096, 512, 10
NCORES = 8
BC = B // NCORES          # 1024 batch rows per core
NF = F // 128             # 32 f-tiles (contraction)
NJ = H // 128             # 4 j-tiles (hidden)
NBC = BC // 512           # 2 moving-operand chunks of 512
NBT = BC // 128           # 8 output b-tiles
NQ = NF // 4              # 8 quads of f-tiles (4 per DMA)

LO_SCALE = 2.0 ** 14      # residual pre-scale; weights carry 2^-14


class _PatchedTileContext(tile.TileContext):
    """Workaround for the walrus build in this container only accepting one
    sem wait on a CTRL-type (Drain) instruction: spread the exit drain's
    per-proc waits across several drains with one wait each."""

    def _drain_and_barrier(self, tick_clock, wait_clock):
        gc = tick_clock.global_clock
        ticks = list(gc)
        nprocs = len(ticks)
        engines = [
            self.nc.sync,
            self.nc.gpsimd,
            self.nc.vector,
            self.nc.scalar,
            self.nc.tensor,
        ]
        # Cheap wait-carriers: one engine NOP per pending proc tick, spread
        # round-robin so the waits resolve in parallel (a DRAIN costs ~1us on
        # some engines; a NOP ~50ns).
        k = 0
        for i, t in enumerate(ticks):
            if t == 0:
                continue
            partial = [0] * nprocs
            partial[i] = t
            inst = engines[k % len(engines)].nop()
            k += 1
            wait_clock.add_sem_waits(
                inst.ins, ScopedClock({None: VectorClock(partial)})
            )
        self.nc.sync.drain()

        self.nc.all_engine_barrier()
        assert self.sems is not None
        popped = self.nc._tile_sem_poison_stack.pop()
        assert popped is self._sem_poison
        if _CLEAR_SEMS:
            # gpsimd-only cleanup (range-clear is a single op there); no
            # closing barrier — each engine halts after its own stream, and
            # NEFF completion waits for all engines anyway.
            self.nc.clear_and_free_semaphores(list(self.sems.allocated().values()))


def _split_waits_json(raw: bytes) -> bytes:
    """The walrus build in this container accepts at most ONE sem wait per
    instruction (bass's own wait_op asserts the same). Tile attaches several.
    Rewrite the serialized BIR: excess waits become standalone EventSemaphore
    wait instructions on the same engine immediately before the instruction —
    semantically identical, since the engine blocks there first."""
    import json as _json

    m = _json.loads(raw)
    ctr = 0
    for fn in m.get("functions", []):
        for bb in fn.get("blocks", []):
            insts = bb.get("instructions", [])
            new_insts = []
            for inst in insts:
                si = inst.get("sync_info")
                waits = si.get("on_wait") or [] if si else []
                if len(waits) > 1:
                    for w in waits[:-1]:
                        new_insts.append(
                            {
                                "debug": inst.get("debug", 0),
                                "engine": inst["engine"],
                                "ins": [],
                                "outs": [],
                                "name": f"WSPLIT-{ctr}",
                                "opcode": "EventSemaphore",
                                "sync_info": {"on_update": [], "on_wait": [w]},
                            }
                        )
                        ctr += 1
                    si["on_wait"] = [waits[-1]]
                new_insts.append(inst)
            bb["instructions"] = new_insts
    return _json.dumps(m).encode()


def _dedup_ldweights_json(raw: bytes) -> bytes:
    """The bass lowering emits one Ldweights per Matmult even when
    consecutive matmuls share the stationary operand (both bc chunks of a
    (f, j) pair). The duplicate loads are pure re-streams of the same
    weights (213ns each for the fp8 DoubleRow pairs, where they gate the
    whole lo pass). Drop any Ldweights identical to the previous one on the
    PE stream with no intervening weight-clobbering op; they carry no sem
    waits/updates (asserted)."""
    import json as _json

    m = _json.loads(raw)
    for fn in m.get("functions", []):
        for bb in fn.get("blocks", []):
            insts = bb.get("instructions", [])
            out = []
            last_key = None
            for inst in insts:
                if inst["engine"] == "PE":
                    op = inst["opcode"]
                    if op == "Ldweights":
                        key = _json.dumps(
                            [
                                inst.get("ins"),
                                inst.get("perf_mode"),
                                inst.get("tile_position"),
                                inst.get("tile_size"),
                                inst.get("is_transpose"),
                            ],
                            sort_keys=True,
                        )
                        si = inst.get("sync_info") or {}
                        clean = not (si.get("on_wait") or si.get("on_update"))
                        if key == last_key and clean:
                            continue  # redundant reload of identical weights
                        last_key = key
                    elif op != "Matmult":
                        last_key = None
                out.append(inst)
            bb["instructions"] = out
    return _json.dumps(m).encode()


def _install_wait_splitter(nc: bass.Bass) -> None:
    orig = nc.to_json_bytes

    def patched():
        return _split_waits_json(_dedup_ldweights_json(orig()))

    nc.to_json_bytes = patched


def build_kernel() -> bass.Bass:
    nc = bass.Bass()
    # Quad-packed streams; per DRAM row (partition line):
    #   xhq: [i=0..3][1024 b]            fp16, 8KB
    #   xlq: [s=0..1][i2=0..1][1024 b]   e4m3, 4KB   (f = q*512+i2*256+s*128+p)
    #   w1h: [i=0..3][512 h]             fp16, 4KB
    #   w1l: [s=0..1][i2=0..1][512 h]    e5m2, 2KB
    xhq = nc.dram_tensor("xhq", [NQ * 128, 4 * BC], F16, kind="ExternalInput")
    xlq = nc.dram_tensor("xlq", [NQ * 128, 2, 2 * BC], E4, kind="ExternalInput")
    w1h = nc.dram_tensor("w1h", [NQ * 128, 4 * H], F16, kind="ExternalInput")
    w1l = nc.dram_tensor("w1l", [NQ * 128, 2, 2 * H], E5, kind="ExternalInput")
    b1p = nc.dram_tensor("b1p", [128, NJ], F32, kind="ExternalInput")
    w2s = nc.dram_tensor("w2s", [128, NJ * C], BF16, kind="ExternalInput")
    b2r = nc.dram_tensor("b2r", [128, C], F32, kind="ExternalInput")
    # packed per-core output [p, bt*10+c]; host reorders to [1024, 10]
    out = nc.dram_tensor("out", [128, NBT * C], F32, kind="ExternalOutput")

    with _PatchedTileContext(nc) as tc:
        with (
            tc.tile_pool(name="consts", bufs=1) as consts,
            tc.tile_pool(name="w1h", bufs=3) as w1h_pool,
            tc.tile_pool(name="w1l", bufs=3) as w1l_pool,
            tc.tile_pool(name="xh", bufs=3) as xh_pool,
            tc.tile_pool(name="xl", bufs=3) as xl_pool,
            tc.tile_pool(name="signh", bufs=NJ * NBC) as signh_pool,
            tc.tile_pool(name="psum", bufs=8, space="PSUM") as psum_pool,
            tc.tile_pool(name="smx", bufs=4) as smx_pool,
        ):
            psumB = [
                [psum_pool.tile([128, 512], F32, name="psB", tag="psB") for _ in range(NBC)]
                for _ in range(NJ)
            ]

            # HAM warmup: keep PE busy while the first tiles land so the
            # 1.2->2.4GHz transition happens during the DMA wait (an idle
            # gap >3.4us re-throttles the clock gate). 15 cold N=512 MMs
            # bridge preamble-end (~7.5us) to first-data (~14us) so the
            # real matmuls start at full clock instead of idling ~3us and
            # re-throttling. Dummy MMs
            # go to bank 0; the first real start=True matmul overwrites.
            warm = consts.tile([128, 640], BF16, name="warm", tag="warm")
            nc.vector.memset(warm[:], 0.0)
            for _ in range(15):
                nc.tensor.matmul(
                    psumB[0][0][:], warm[:, :128], warm[:, 128:640],
                    start=True, stop=True,
                )

            def quad_in(q):
                """DMA one quad's four streams; returns (w1h, w1l, xh, xl) tiles.
                x streams ride the SP HWDGE ring, weights ride the ACT ring,
                so the two drain in parallel."""
                rows = slice(q * 128, (q + 1) * 128)
                wh = w1h_pool.tile([128, 4 * H], F16, name="wh", tag="wh")
                xh = xh_pool.tile([128, 4 * BC], F16, name="xh", tag="xh")
                if q == 0:
                    # startup: the first matmul needs only wh[:, :128j] and
                    # xh[:, :512b]; put exactly those two small transfers at
                    # the head of the sync ring (each DMA issue costs the
                    # engine ~0.6-1.3us, so fewer+targeted beats many pieces)
                    nc.sync.dma_start(wh[:, 0:H], w1h[rows, 0:H])
                    nc.sync.dma_start(xh[:, 0:BC], xhq[rows, 0:BC])
                    nc.sync.dma_start(xh[:, BC:4 * BC], xhq[rows, BC:4 * BC])
                    nc.scalar.dma_start(wh[:, H:4 * H], w1h[rows, H:4 * H])
                else:
                    nc.scalar.dma_start(wh[:], w1h[rows, :])
                    nc.sync.dma_start(xh[:], xhq[rows, :])
                wl = w1l_pool.tile([128, 2, 2 * H], E5, name="wl", tag="wl")
                nc.scalar.dma_start(wl[:], w1l[rows, :, :])
                xl = xl_pool.tile([128, 2, 2 * BC], E4, name="xl", tag="xl")
                nc.sync.dma_start(xl[:], xlq[rows, :, :])
                return wh, wl, xh, xl

            def hi_mm(wh, xh, i, j, bc, start, self_load=True):
                mm = nc.tensor.matmul(
                    psumB[j][bc][:],
                    wh[:, i * H + j * 128:i * H + (j + 1) * 128],
                    xh[:, i * BC + bc * 512:i * BC + (bc + 1) * 512],
                    start=start, stop=False, skip_group_check=True,
                )
                if not self_load:
                    mm.ins.ldweights = False

            def lo_mm(wl, xl, i2, j, bc, stop, self_load=True):
                mm = nc.tensor.matmul(
                    psumB[j][bc][:],
                    wl[:, :, i2 * H + j * 128:i2 * H + (j + 1) * 128],
                    xl[:, :, i2 * BC + bc * 512:i2 * BC + (bc + 1) * 512],
                    start=False, stop=stop, skip_group_check=True,
                    perf_mode=DR,
                )
                if not self_load:
                    mm.ins.ldweights = False

            # ---- phase 1: quads 0..NQ-2, f-major over all 8 banks.
            # Per quad: all hi matmuls, then one lo block. The lo blocks
            # consume only SBUF-resident data, so they double as catch-up
            # slots for the x DMA ring; grouping them away starves it. ----
            b1_t = w2_t = b2_t = None
            for q in range(NQ - 1):
                if q == 0:
                    with tc.high_priority():
                        wh, wl, xh, xl = quad_in(q)
                else:
                    wh, wl, xh, xl = quad_in(q)
                if q == 0:
                    # constants: packed, one DMA each, after the first quad's
                    # stream DMAs so they stay off the startup critical path
                    b1_t = consts.tile([128, NJ], F32, name="b1t", tag="b1t")
                    nc.scalar.dma_start(b1_t[:], b1p[:, :])
                    w2_t = consts.tile([128, NJ * C], BF16, name="w2t", tag="w2t")
                    nc.scalar.dma_start(w2_t[:], w2s[:, :])
                    b2_t = consts.tile([128, C], F32, name="b2", tag="b2")
                    nc.scalar.dma_start(b2_t[:], b2r[:, :])
                for i in range(4):
                    for j in range(NJ):
                        for bc in range(NBC):
                            hi_mm(wh, xh, i, j, bc, start=(q == 0 and i == 0))
                for i2 in range(2):
                    for j in range(NJ):
                        for bc in range(NBC):
                            lo_mm(wl, xl, i2, j, bc, stop=False)

            # ---- phase 2: last quad bank-major; sign/mm2/softmax overlap ----
            wh, wl, xh, xl = quad_in(NQ - 1)
            signh = [[None] * NBC for _ in range(NJ)]
            collect = smx_pool.tile([128, NBT * C], F32, name="collect", tag="collect")
            for bc in range(NBC):
                for j in range(NJ):
                    for i in range(4):
                        hi_mm(wh, xh, i, j, bc, start=False)
                for j in range(NJ):
                    for i2 in range(2):
                        lo_mm(wl, xl, i2, j, bc, stop=(i2 == 1))
                    s = signh_pool.tile([128, 512], BF16, name="signh", tag="signh")
                    nc.scalar.sign(s[:], psumB[j][bc][:], bias=b1_t[:, j:j + 1])
                    signh[j][bc] = s
                for bt in range(bc * 4, bc * 4 + 4):
                    col = (bt % 4) * 128
                    ps2 = psum_pool.tile([128, C], F32, name="psD", tag="psB")
                    for j in range(NJ):
                        nc.tensor.matmul(
                            ps2[:],
                            signh[j][bc][:, col:col + 128],
                            w2_t[:, j * C:(j + 1) * C],
                            start=(j == 0),
                            stop=(j == NJ - 1),
                        )
                    # softmax with ACT only on the exp; everything else DVE
                    logits = smx_pool.tile([128, C], F32, name="logits", tag="logits")
                    nc.vector.tensor_add(logits[:], ps2[:], b2_t[:])
                    negmax = smx_pool.tile([128, 1], F32, name="negmax", tag="negmax")
                    nc.vector.reduce_max(
                        negmax[:], logits[:], axis=mybir.AxisListType.X, negate=True
                    )
                    e = smx_pool.tile([128, C], F32, name="e", tag="e")
                    nc.scalar.activation(
                        e[:],
                        logits[:],
                        mybir.ActivationFunctionType.Exp,
                        bias=negmax[:],
                    )
                    ssum = smx_pool.tile([128, 1], F32, name="ssum", tag="ssum")
                    nc.vector.tensor_reduce(
                        ssum[:], e[:], axis=mybir.AxisListType.X,
                        op=mybir.AluOpType.add,
                    )
                    rsum = smx_pool.tile([128, 1], F32, name="rsum", tag="rsum")
                    nc.vector.reciprocal(rsum[:], ssum[:])
                    nc.vector.tensor_scalar_mul(
                        collect[:, bt * C:(bt + 1) * C], e[:], rsum[:]
                    )

            # per-chunk output DMAs; host unpacks [p, bt*10+c] -> [bt*128+p, c].
            # bc0's half leaves while bc1 computes, so only a 20KB transfer
            # trails the final softmax
            half = 4 * C
            nc.sync.dma_start(out[:, 0:half], collect[:, 0:half])
            nc.sync.dma_start(out[:, half:2 * half], collect[:, half:2 * half])

    _install_wait_splitter(nc)
    return nc


_cached_nc = None


def _get_nc() -> bass.Bass:
    global _cached_nc
    if _cached_nc is None:
        _cached_nc = build_kernel()
    return _cached_nc


def kernel(inputs, W1, b1, W2, b2):
    x = np.ascontiguousarray(np.asarray(inputs, dtype=np.float32))
    W1 = np.asarray(W1, dtype=np.float32)
    b1 = np.asarray(b1, dtype=np.float32)
    W2 = np.asarray(W2, dtype=np.float32)
    b2 = np.asarray(b2, dtype=np.float32)

    sW1 = np.where(W1 >= 0, np.float32(1.0), np.float32(-1.0))
    # hi: [4096, 512] fp16 -> quad-packed [NQ*128, 4*512]
    w1h_pack = np.ascontiguousarray(
        sW1.astype(np.float16)
        .reshape(NQ, 4, 128, H).transpose(0, 2, 1, 3).reshape(NQ * 128, 4 * H)
    )
    # lo stationary: +-2^-14 e5m2, pair-packed [NQ*128, 2(s), 2(i2)*512]
    w1l_pack = np.ascontiguousarray(
        (sW1 * np.float32(1.0 / LO_SCALE)).astype(ml_dtypes.float8_e5m2)
        .reshape(NQ, 2, 2, 128, H).transpose(0, 3, 2, 1, 4)
        .reshape(NQ * 128, 2, 2 * H)
    )
    b1_pack = np.ascontiguousarray(b1.reshape(NJ, 128).T)
    w2_pack = np.ascontiguousarray(
        np.where(W2 >= 0, np.float32(1.0), np.float32(-1.0))
        .astype(ml_dtypes.bfloat16)
        .reshape(NJ, 128, C).transpose(1, 0, 2).reshape(128, NJ * C)
    )
    b2_rep = np.ascontiguousarray(np.broadcast_to(b2.reshape(1, C), (128, C)))

    in_maps = []
    for c in range(NCORES):
        xc_t = np.ascontiguousarray(x[c * BC:(c + 1) * BC, :].T)  # [F, BC]
        hi = xc_t.astype(np.float16)
        lo = ((xc_t - hi.astype(np.float32)) * np.float32(LO_SCALE)).astype(
            ml_dtypes.float8_e4m3
        )
        xh_pack = np.ascontiguousarray(
            hi.reshape(NQ, 4, 128, BC).transpose(0, 2, 1, 3).reshape(NQ * 128, 4 * BC)
        )
        xl_pack = np.ascontiguousarray(
            lo.reshape(NQ, 2, 2, 128, BC).transpose(0, 3, 2, 1, 4)
            .reshape(NQ * 128, 2, 2 * BC)
        )
        in_maps.append(
            {
                "xhq": xh_pack,
                "xlq": xl_pack,
                "w1h": w1h_pack,
                "w1l": w1l_pack,
                "w2s": w2_pack,
                "b1p": b1_pack,
                "b2r": b2_rep,
            }
        )

    nc = _get_nc()
    res = run_bass_kernel_spmd(nc, in_maps, core_ids=list(range(NCORES)))
    global last_results
    last_results = res
    parts = []
    for c in range(NCORES):
        oc = res.results[c]["out"]  # [128, NBT*C]
        parts.append(
            oc.reshape(128, NBT, C).transpose(1, 0, 2).reshape(BC, C)
        )
    return np.concatenate(parts, axis=0).astype(np.float32)


last_results = None


# revision 24
# speedup vs baseline: 1.0117x; 1.0117x over previous
"""Binarized-MLP (BNN) kernel for Trainium2, data-parallel over batch on 8 NeuronCores.

Reference computation:
    h      = x @ sign(W1) + b1          x:[8192,4096] W1:[4096,512]
    logits = sign(h) @ sign(W2) + b2    W2:[512,10]
    out    = softmax(logits)            [8192,10]

Device strategy (per core, batch shard of 1024 rows):
  - Mixed-precision first matmul in ~1.5 passes instead of 2:
      hi pass: x in fp16 (11 sig bits, exact products in the PE's FP22
               pipeline), stationary sign(W1) in fp16 (+-1 exact).
      lo pass: residual r = x - fp16(x), scaled by 2^14 and cast to e4m3,
               via fp8 DoubleRow matmuls (2 contraction rows per PE cell,
               ~0.5 cycles/row). The 2^-14 descale is baked into the
               stationary operand (+-2^-14, exactly representable as an
               e5m2 NORMAL), so lo products are +-r and accumulate into
               the SAME fp32 PSUM banks as the hi pass - no combine step.
    Total x precision ~15 bits -> end-to-end rel err ~1.2e-2 (measured,
    deterministic inputs; gate is 2e-2). Sign flips of h drive the error;
    fp16-only (11 bits) measures 5e-2 and fails, hi/lo bf16 (baseline)
    measures 2.8e-4 but costs a full 2nd pass.
  - Layout: stationary = sign(W1) f-tile [128f x 128j], moving = xT f-tile
    [128f x 512b] -> PSUM [128j x 512b]; all 8 PSUM banks hold the full
    per-core h [512 x 1024] and accumulate across the 32 f-tiles.
  - Inputs host-packed so four f-tiles arrive per DMA (quads); the last
    quad runs bank-major so sign(h)/second-matmul/softmax overlap it.
  - sign(h)+b1 fused into one ScalarE Sign-activation (bias=b1) straight
    out of PSUM into bf16 SBUF tiles laid out [j, b] for the 2nd matmul.
  - Second matmul: stationary = sign(h) [128j x 128b], moving = sign(W2)
    [128j x 10] accumulated over 4 j-tiles -> PSUM [128b x 10].
  - Softmax on [128b, 10] tiles: add b2, reduce_max(negate) -> Exp with
    per-row bias and fused row-sum, Ln, subtract, Exp -> packed DMA out.
"""

import numpy as np
import ml_dtypes

import concourse.bass as bass
import concourse.bass_utils as _bass_utils
import concourse.tile as tile
from concourse import mybir
from concourse.bass_utils import run_bass_kernel_spmd
from bass_rust import ScopedClock, VectorClock

_CLEAR_SEMS = True

BF16 = mybir.dt.bfloat16
F16 = mybir.dt.float16
F32 = mybir.dt.float32
E4 = mybir.dt.float8e4
E5 = mybir.dt.float8e5
DR = mybir.MatmulPerfMode.DoubleRow

B, F, H, C = 8192, 4096, 512, 10
NCORES = 8
BC = B // NCORES          # 1024 batch rows per core
NF = F // 128             # 32 f-tiles (contraction)
NJ = H // 128             # 4 j-tiles (hidden)
NBC = BC // 512           # 2 moving-operand chunks of 512
NBT = BC // 128           # 8 output b-tiles
NQ = NF // 4              # 8 quads of f-tiles (4 per DMA)

LO_SCALE = 2.0 ** 14      # residual pre-scale; weights carry 2^-14


class _PatchedTileContext(tile.TileContext):
    """Workaround for the walrus build in this container only accepting one
    sem wait on a CTRL-type (Drain) instruction: spread the exit drain's
    per-proc waits across several drains with one wait each."""

    def _drain_and_barrier(self, tick_clock, wait_clock):
        gc = tick_clock.global_clock
        ticks = list(gc)
        nprocs = len(ticks)
        engines = [
            self.nc.sync,
            self.nc.gpsimd,
            self.nc.vector,
            self.nc.scalar,
            self.nc.tensor,
        ]
        # Cheap wait-carriers: one engine NOP per pending proc tick, spread
        # round-robin so the waits resolve in parallel (a DRAIN costs ~1us on
        # some engines; a NOP ~50ns).
        k = 0
        for i, t in enumerate(ticks):
            if t == 0:
                continue
            partial = [0] * nprocs
            partial[i] = t
            inst = engines[k % len(engines)].nop()
            k += 1
            wait_clock.add_sem_waits(
                inst.ins, ScopedClock({None: VectorClock(partial)})
            )
        self.nc.sync.drain()

        self.nc.all_engine_barrier()
        assert self.sems is not None
        popped = self.nc._tile_sem_poison_stack.pop()
        assert popped is self._sem_poison
        if _CLEAR_SEMS:
            # gpsimd-only cleanup (range-clear is a single op there); no
            # closing barrier — each engine halts after its own stream, and
            # NEFF completion waits for all engines anyway.
            self.nc.clear_and_free_semaphores(list(self.sems.allocated().values()))


def _split_waits_json(raw: bytes) -> bytes:
    """The walrus build in this container accepts at most ONE sem wait per
    instruction (bass's own wait_op asserts the same). Tile attaches several.
    Rewrite the serialized BIR: excess waits become standalone EventSemaphore
    wait instructions on the same engine immediately before the instruction —
    semantically identical, since the engine blocks there first."""
    import json as _json

    m = _json.loads(raw)
    ctr = 0
    for fn in m.get("functions", []):
        for bb in fn.get("blocks", []):
            insts = bb.get("instructions", [])
            new_insts = []
            for inst in insts:
                si = inst.get("sync_info")
                waits = si.get("on_wait") or [] if si else []
                if len(waits) > 1:
                    for w in waits[:-1]:
                        new_insts.append(
                            {
                                "debug": inst.get("debug", 0),
                                "engine": inst["engine"],
                                "ins": [],
                                "outs": [],
                                "name": f"WSPLIT-{ctr}",
                                "opcode": "EventSemaphore",
                                "sync_info": {"on_update": [], "on_wait": [w]},
                            }
                        )
                        ctr += 1
                    si["on_wait"] = [waits[-1]]
                new_insts.append(inst)
            bb["instructions"] = new_insts
    return _json.dumps(m).encode()


def _dedup_ldweights_json(raw: bytes) -> bytes:
    """The bass lowering emits one Ldweights per Matmult even when
    consecutive matmuls share the stationary operand (both bc chunks of a
    (f, j) pair). The duplicate loads are pure re-streams of the same
    weights (213ns each for the fp8 DoubleRow pairs, where they gate the
    whole lo pass). Drop any Ldweights identical to the previous one on the
    PE stream with no intervening weight-clobbering op; they carry no sem
    waits/updates (asserted)."""
    import json as _json

    m = _json.loads(raw)
    for fn in m.get("functions", []):
        for bb in fn.get("blocks", []):
            insts = bb.get("instructions", [])
            out = []
            last_key = None
            for inst in insts:
                if inst["engine"] == "PE":
                    op = inst["opcode"]
                    if op == "Ldweights":
                        key = _json.dumps(
                            [
                                inst.get("ins"),
                                inst.get("perf_mode"),
                                inst.get("tile_position"),
                                inst.get("tile_size"),
                                inst.get("is_transpose"),
                            ],
                            sort_keys=True,
                        )
                        si = inst.get("sync_info") or {}
                        clean = not (si.get("on_wait") or si.get("on_update"))
                        if key == last_key and clean:
                            continue  # redundant reload of identical weights
                        last_key = key
                    elif op != "Matmult":
                        last_key = None
                out.append(inst)
            bb["instructions"] = out
    return _json.dumps(m).encode()


def _install_wait_splitter(nc: bass.Bass) -> None:
    orig = nc.to_json_bytes

    def patched():
        return _split_waits_json(_dedup_ldweights_json(orig()))

    nc.to_json_bytes = patched


def build_kernel() -> bass.Bass:
    nc = bass.Bass()
    # Quad-packed streams; per DRAM row (partition line):
    #   xhq: [i=0..3][1024 b]            fp16, 8KB
    #   xlq: [s=0..1][i2=0..1][1024 b]   e4m3, 4KB   (f = q*512+i2*256+s*128+p)
    #   w1h: [i=0..3][512 h]             fp16, 4KB
    #   w1l: [s=0..1][i2=0..1][512 h]    e5m2, 2KB
    xhq = nc.dram_tensor("xhq", [NQ * 128, 4 * BC], F16, kind="ExternalInput")
    xlq = nc.dram_tensor("xlq", [NQ * 128, 2, 2 * BC], E4, kind="ExternalInput")
    w1h = nc.dram_tensor("w1h", [NQ * 128, 4 * H], F16, kind="ExternalInput")
    w1l = nc.dram_tensor("w1l", [NQ * 128, 2, 2 * H], E5, kind="ExternalInput")
    b1p = nc.dram_tensor("b1p", [128, NJ], F32, kind="ExternalInput")
    w2s = nc.dram_tensor("w2s", [128, NJ * C], BF16, kind="ExternalInput")
    b2r = nc.dram_tensor("b2r", [128, C], F32, kind="ExternalInput")
    # packed per-core output [p, bt*10+c]; host reorders to [1024, 10]
    out = nc.dram_tensor("out", [128, NBT * C], F32, kind="ExternalOutput")

    with _PatchedTileContext(nc) as tc:
        with (
            tc.tile_pool(name="consts", bufs=1) as consts,
            tc.tile_pool(name="w1h", bufs=3) as w1h_pool,
            tc.tile_pool(name="w1l", bufs=3) as w1l_pool,
            tc.tile_pool(name="xh", bufs=3) as xh_pool,
            tc.tile_pool(name="xl", bufs=3) as xl_pool,
            tc.tile_pool(name="signh", bufs=NJ * NBC) as signh_pool,
            tc.tile_pool(name="psum", bufs=8, space="PSUM") as psum_pool,
            tc.tile_pool(name="smx", bufs=4) as smx_pool,
        ):
            psumB = [
                [psum_pool.tile([128, 512], F32, name="psB", tag="psB") for _ in range(NBC)]
                for _ in range(NJ)
            ]

            # HAM warmup: keep PE busy while the first tiles land so the
            # 1.2->2.4GHz transition happens during the DMA wait (an idle
            # gap >3.4us re-throttles the clock gate). 15 cold N=512 MMs
            # bridge preamble-end (~7.5us) to first-data (~14us) so the
            # real matmuls start at full clock instead of idling ~3us and
            # re-throttling. Dummy MMs
            # go to bank 0; the first real start=True matmul overwrites.
            warm = consts.tile([128, 640], BF16, name="warm", tag="warm")
            nc.vector.memset(warm[:], 0.0)
            for _ in range(15):
                nc.tensor.matmul(
                    psumB[0][0][:], warm[:, :128], warm[:, 128:640],
                    start=True, stop=True,
                )

            def quad_in(q):
                """DMA one quad's four streams; returns (w1h, w1l, xh, xl) tiles.
                x streams ride the SP HWDGE ring, weights ride the ACT ring,
                so the two drain in parallel."""
                rows = slice(q * 128, (q + 1) * 128)
                wh = w1h_pool.tile([128, 4 * H], F16, name="wh", tag="wh")
                xh = xh_pool.tile([128, 4 * BC], F16, name="xh", tag="xh")
                if q == 0:
                    # startup: the first matmul needs only wh[:, :128j] and
                    # xh[:, :512b]; put exactly those two small transfers at
                    # the head of the sync ring (each DMA issue costs the
                    # engine ~0.6-1.3us, so fewer+targeted beats many pieces)
                    nc.sync.dma_start(wh[:, 0:H], w1h[rows, 0:H])
                    nc.sync.dma_start(xh[:, 0:BC], xhq[rows, 0:BC])
                    nc.sync.dma_start(xh[:, BC:4 * BC], xhq[rows, BC:4 * BC])
                    nc.scalar.dma_start(wh[:, H:4 * H], w1h[rows, H:4 * H])
                elif q % 2 == 0:
                    nc.scalar.dma_start(wh[:], w1h[rows, :])
                    nc.sync.dma_start(xh[:], xhq[rows, :])
                else:
                    # alternate the 2MB x stream between the two HWDGE rings
                    # so consecutive quads load concurrently: the startup
                    # region is otherwise bound by a single ring's ramp
                    nc.sync.dma_start(wh[:], w1h[rows, :])
                    nc.scalar.dma_start(xh[:], xhq[rows, :])
                wl = w1l_pool.tile([128, 2, 2 * H], E5, name="wl", tag="wl")
                nc.scalar.dma_start(wl[:], w1l[rows, :, :])
                xl = xl_pool.tile([128, 2, 2 * BC], E4, name="xl", tag="xl")
                nc.sync.dma_start(xl[:], xlq[rows, :, :])
                return wh, wl, xh, xl

            def hi_mm(wh, xh, i, j, bc, start, self_load=True):
                mm = nc.tensor.matmul(
                    psumB[j][bc][:],
                    wh[:, i * H + j * 128:i * H + (j + 1) * 128],
                    xh[:, i * BC + bc * 512:i * BC + (bc + 1) * 512],
                    start=start, stop=False, skip_group_check=True,
                )
                if not self_load:
                    mm.ins.ldweights = False

            def lo_mm(wl, xl, i2, j, bc, stop, self_load=True):
                mm = nc.tensor.matmul(
                    psumB[j][bc][:],
                    wl[:, :, i2 * H + j * 128:i2 * H + (j + 1) * 128],
                    xl[:, :, i2 * BC + bc * 512:i2 * BC + (bc + 1) * 512],
                    start=False, stop=stop, skip_group_check=True,
                    perf_mode=DR,
                )
                if not self_load:
                    mm.ins.ldweights = False

            # ---- phase 1: quads 0..NQ-2, f-major over all 8 banks.
            # Per quad: all hi matmuls, then one lo block. The lo blocks
            # consume only SBUF-resident data, so they double as catch-up
            # slots for the x DMA ring; grouping them away starves it. ----
            b1_t = w2_t = b2_t = None
            for q in range(NQ - 1):
                if q == 0:
                    with tc.high_priority():
                        wh, wl, xh, xl = quad_in(q)
                else:
                    wh, wl, xh, xl = quad_in(q)
                if q == 0:
                    # constants: packed, one DMA each, after the first quad's
                    # stream DMAs so they stay off the startup critical path
                    b1_t = consts.tile([128, NJ], F32, name="b1t", tag="b1t")
                    nc.scalar.dma_start(b1_t[:], b1p[:, :])
                    w2_t = consts.tile([128, NJ * C], BF16, name="w2t", tag="w2t")
                    nc.scalar.dma_start(w2_t[:], w2s[:, :])
                    b2_t = consts.tile([128, C], F32, name="b2", tag="b2")
                    nc.scalar.dma_start(b2_t[:], b2r[:, :])
                for i in range(4):
                    for j in range(NJ):
                        for bc in range(NBC):
                            hi_mm(wh, xh, i, j, bc, start=(q == 0 and i == 0))
                for i2 in range(2):
                    for j in range(NJ):
                        for bc in range(NBC):
                            lo_mm(wl, xl, i2, j, bc, stop=False)

            # ---- phase 2: last quad bank-major; sign/mm2/softmax overlap ----
            wh, wl, xh, xl = quad_in(NQ - 1)
            signh = [[None] * NBC for _ in range(NJ)]
            collect = smx_pool.tile([128, NBT * C], F32, name="collect", tag="collect")
            for bc in range(NBC):
                for j in range(NJ):
                    for i in range(4):
                        hi_mm(wh, xh, i, j, bc, start=False)
                for j in range(NJ):
                    for i2 in range(2):
                        lo_mm(wl, xl, i2, j, bc, stop=(i2 == 1))
                    s = signh_pool.tile([128, 512], BF16, name="signh", tag="signh")
                    nc.scalar.sign(s[:], psumB[j][bc][:], bias=b1_t[:, j:j + 1])
                    signh[j][bc] = s
                for bt in range(bc * 4, bc * 4 + 4):
                    col = (bt % 4) * 128
                    ps2 = psum_pool.tile([128, C], F32, name="psD", tag="psB")
                    for j in range(NJ):
                        nc.tensor.matmul(
                            ps2[:],
                            signh[j][bc][:, col:col + 128],
                            w2_t[:, j * C:(j + 1) * C],
                            start=(j == 0),
                            stop=(j == NJ - 1),
                        )
                    # softmax with ACT only on the exp; everything else DVE
                    logits = smx_pool.tile([128, C], F32, name="logits", tag="logits")
                    nc.vector.tensor_add(logits[:], ps2[:], b2_t[:])
                    negmax = smx_pool.tile([128, 1], F32, name="negmax", tag="negmax")
                    nc.vector.reduce_max(
                        negmax[:], logits[:], axis=mybir.AxisListType.X, negate=True
                    )
                    e = smx_pool.tile([128, C], F32, name="e", tag="e")
                    nc.scalar.activation(
                        e[:],
                        logits[:],
                        mybir.ActivationFunctionType.Exp,
                        bias=negmax[:],
                    )
                    ssum = smx_pool.tile([128, 1], F32, name="ssum", tag="ssum")
                    nc.vector.tensor_reduce(
                        ssum[:], e[:], axis=mybir.AxisListType.X,
                        op=mybir.AluOpType.add,
                    )
                    rsum = smx_pool.tile([128, 1], F32, name="rsum", tag="rsum")
                    nc.vector.reciprocal(rsum[:], ssum[:])
                    nc.vector.tensor_scalar_mul(
                        collect[:, bt * C:(bt + 1) * C], e[:], rsum[:]
                    )

            # per-chunk output DMAs; host unpacks [p, bt*10+c] -> [bt*128+p, c].
            # bc0's half leaves while bc1 computes, so only a 20KB transfer
            # trails the final softmax
            half = 4 * C
            nc.sync.dma_start(out[:, 0:half], collect[:, 0:half])
            nc.sync.dma_start(out[:, half:2 * half], collect[:, half:2 * half])

    _install_wait_splitter(nc)
    return nc


_cached_nc = None


def _get_nc() -> bass.Bass:
    global _cached_nc
    if _cached_nc is None:
        _cached_nc = build_kernel()
    return _cached_nc


def kernel(inputs, W1, b1, W2, b2):
    x = np.ascontiguousarray(np.asarray(inputs, dtype=np.float32))
    W1 = np.asarray(W1, dtype=np.float32)
    b1 = np.asarray(b1, dtype=np.float32)
    W2 = np.asarray(W2, dtype=np.float32)
    b2 = np.asarray(b2, dtype=np.float32)

    sW1 = np.where(W1 >= 0, np.float32(1.0), np.float32(-1.0))
    # hi: [4096, 512] fp16 -> quad-packed [NQ*128, 4*512]
    w1h_pack = np.ascontiguousarray(
        sW1.astype(np.float16)
        .reshape(NQ, 4, 128, H).transpose(0, 2, 1, 3).reshape(NQ * 128, 4 * H)
    )
    # lo stationary: +-2^-14 e5m2, pair-packed [NQ*128, 2(s), 2(i2)*512]
    w1l_pack = np.ascontiguousarray(
        (sW1 * np.float32(1.0 / LO_SCALE)).astype(ml_dtypes.float8_e5m2)
        .reshape(NQ, 2, 2, 128, H).transpose(0, 3, 2, 1, 4)
        .reshape(NQ * 128, 2, 2 * H)
    )
    b1_pack = np.ascontiguousarray(b1.reshape(NJ, 128).T)
    w2_pack = np.ascontiguousarray(
        np.where(W2 >= 0, np.float32(1.0), np.float32(-1.0))
        .astype(ml_dtypes.bfloat16)
        .reshape(NJ, 128, C).transpose(1, 0, 2).reshape(128, NJ * C)
    )
    b2_rep = np.ascontiguousarray(np.broadcast_to(b2.reshape(1, C), (128, C)))

    in_maps = []
    for c in range(NCORES):
        xc_t = np.ascontiguousarray(x[c * BC:(c + 1) * BC, :].T)  # [F, BC]
        hi = xc_t.astype(np.float16)
        lo = ((xc_t - hi.astype(np.float32)) * np.float32(LO_SCALE)).astype(
            ml_dtypes.float8_e4m3
        )
        xh_pack = np.ascontiguousarray(
            hi.reshape(NQ, 4, 128, BC).transpose(0, 2, 1, 3).reshape(NQ * 128, 4 * BC)
        )
        xl_pack = np.ascontiguousarray(
            lo.reshape(NQ, 2, 2, 128, BC).transpose(0, 3, 2, 1, 4)
            .reshape(NQ * 128, 2, 2 * BC)
        )
        in_maps.append(
            {
                "xhq": xh_pack,
                "xlq": xl_pack,
                "w1h": w1h_pack,
                "w1l": w1l_pack,
                "w2s": w2_pack,
                "b1p": b1_pack,
                "b2r": b2_rep,
            }
        )

    nc = _get_nc()
    res = run_bass_kernel_spmd(nc, in_maps, core_ids=list(range(NCORES)))
    global last_results
    last_results = res
    parts = []
    for c in range(NCORES):
        oc = res.results[c]["out"]  # [128, NBT*C]
        parts.append(
            oc.reshape(128, NBT, C).transpose(1, 0, 2).reshape(BC, C)
        )
    return np.concatenate(parts, axis=0).astype(np.float32)


last_results = None
